# revision 38
# baseline (speedup 1.0000x reference)
"""TRN2 Bass kernel for nn_ActionDecoderCrossAttention.

Sharding: 8 cores = 2 (batch) x 4 (head-groups of 4 heads).
Per core: RMSNorm(x) -> qT (f32r), caT via PE-transpose (fp32->f32r),
kT/v projections, scoresT = k @ qT per head (f32r), exp on ACT with mask bias
(no max-subtraction: |score| < 70 guaranteed by construction), PV with an
appended ones-column (M=65) so softmax denominators accumulate in the same
matmul, normalize, AllToAll to redistribute attn^T by L-slice, final
projection vs full Wo (bf16), output y slice [256, 1024] fp32.

Self-contained: hardcodes all shapes; builds the Bass program once per process.
"""

import numpy as np

import concourse.bass as bass
import concourse.mybir as mybir
import concourse.tile as tile
from concourse import bacc
from concourse.bass_utils import run_bass_kernel_spmd
from concourse.masks import make_identity

F32 = mybir.dt.float32
F32R = mybir.dt.float32r
BF16 = mybir.dt.bfloat16
AF = mybir.ActivationFunctionType
ALU = mybir.AluOpType

B, L, S, D, CA, H, DH = 2, 1024, 4096, 1024, 1024, 16, 64
N_CORES = 8
G = 4               # head-groups (tensor-parallel degree per batch)
HPC = H // G        # heads per core = 4
CS = HPC * DH       # per-core CA shard = 256
LSL = L // G        # output L-slice per core = 256
NKT = D // 128      # 8 contraction tiles
NLT = L // 128      # 8 L tiles
NST = S // 128      # 32 S tiles
SCH = 512           # ca processing chunk (s dimension)
NCH = S // SCH      # 8 chunks
REPLICA_GROUPS = [[0, 1, 2, 3], [4, 5, 6, 7]]


def build_program(repeat=1, debug=False):
    nc = bacc.Bacc("TRN2", target_bir_lowering=False, debug=False,
                   num_devices=N_CORES)
    dbg = {}
    if debug:
        dbg["qT"] = nc.dram_tensor("dbg_qT", [128, 2, L], F32R, kind="ExternalOutput")
        dbg["probs"] = nc.dram_tensor("dbg_probs", [128, 1024], BF16, kind="ExternalOutput")
        dbg["dsb"] = nc.dram_tensor("dbg_dsb", [128, 2048], F32, kind="ExternalOutput")
        dbg["rsb"] = nc.dram_tensor("dbg_rsb", [128, 1024], F32R, kind="ExternalOutput")
        dbg["dbc"] = nc.dram_tensor("dbg_dbc", [64, 512], F32, kind="ExternalOutput")
        dbg["attn0"] = nc.dram_tensor("dbg_attn0", [64, L], BF16, kind="ExternalOutput")
        dbg["xnT"] = nc.dram_tensor("dbg_xnT", [128, NKT, L], F32R, kind="ExternalOutput")
        dbg["a2a"] = nc.dram_tensor("dbg_a2a", [N_CORES, CS, LSL], BF16, kind="ExternalOutput")

    bsel_d = nc.dram_tensor("bsel", [64, 2], F32, kind="ExternalInput")
    x_d = nc.dram_tensor("x", [L, D], F32, kind="ExternalInput")
    ca_d = nc.dram_tensor("ca", [S, CA], F32, kind="ExternalInput")
    mask_d = nc.dram_tensor("mask", [32, 128], F32, kind="ExternalInput")
    scale_d = nc.dram_tensor("scale", [NKT, 128], F32, kind="ExternalInput")
    wq_d = nc.dram_tensor("wq", [D, CS], F32, kind="ExternalInput")
    wk_d = nc.dram_tensor("wk", [CA, CS], F32, kind="ExternalInput")
    wv_d = nc.dram_tensor("wv", [CA, CS], F32, kind="ExternalInput")
    wo_d = nc.dram_tensor("wo", [CA, D], F32, kind="ExternalInput")
    y_d = nc.dram_tensor("y", [LSL, D], F32, kind="ExternalOutput")

    with tile.TileContext(nc) as tc:
        with (
            tc.tile_pool(name="persist", bufs=1) as pp,
            tc.tile_pool(name="dram", bufs=1, space="DRAM") as dp,
        ):
            a2a_in = dp.tile([N_CORES, CS, LSL], BF16, tag="a2a_in")
            a2a_out = dp.tile([N_CORES, CS, LSL], BF16, tag="a2a_out")
            bsel_sb = pp.tile([64, 2], F32, tag="bsel_sb")
            nc.sync.dma_start(bsel_sb[:], bsel_d.ap())
            ident = pp.tile([128, 128], F32, tag="ident")
            make_identity(nc, ident[:])
            ident_r = pp.tile([128, 128], F32R, tag="ident_r")
            nc.vector.tensor_copy(ident_r[:], ident[:])
            # persistent sbuf tensors
            prep_T = pp.tile([128, 40], F32, tag="prepT")   # cols 0-31 exp bias, 32-39 scale
            wq_r = pp.tile([128, NKT, CS], F32R, tag="wq_r")
            wk_r = pp.tile([128, NKT, CS], F32R, tag="wk_r")
            wv_r = pp.tile([128, NKT, CS], F32R, tag="wv_r")
            wo_bf = pp.tile([128, NKT, D], BF16, tag="wo_bf")
            qT = pp.tile([128, 2, L], F32R, tag="qT")
            kT = [pp.tile([128, 2, SCH], F32R, tag=f"kT{c}", name=f"kT{c}")
                  for c in range(NCH)]
            v_sb = [pp.tile([128, SCH // 128, HPC * 65], BF16, tag=f"v_sb{c}",
                            name=f"v_sb{c}") for c in range(NCH)]
            ones_r = pp.tile([128, 64], F32R, tag="ones_r")
            ones_f = pp.tile([128, 64], F32, tag="ones_f")
            nc.vector.memset(ones_f[:], 1.0)
            nc.vector.tensor_copy(ones_r[:], ones_f[:])

            for _rep in range(repeat):
                _emit_body(nc, tc, pp, (ident, ident_r), prep_T, wq_r, wk_r, wv_r, wo_bf,
                           qT, kT, v_sb, ones_r, bsel_sb,
                           x_d, ca_d, mask_d, scale_d, wq_d, wk_d, wv_d, wo_d,
                           y_d, a2a_in, a2a_out, dbg)

    nc.finalize()
    return nc


def _emit_body(nc, tc, pp, idents, prep_T, wq_r, wk_r, wv_r, wo_bf,
               qT, kT, v_sb, ones_r, bsel_sb,
               x_d, ca_d, mask_d, scale_d, wq_d, wk_d, wv_d, wo_d,
               y_d, a2a_in, a2a_out, dbg=None):
    dbg = dbg or {}
    ident, ident_r = idents
    # ---------------- stage A: prep (mask bias, scale, weights) ----------------
    with (
        tc.tile_pool(name="stageA", bufs=2) as sa,
        tc.tile_pool(name="psA", bufs=2, space="PSUM") as psa,
    ):
        prep_in = sa.tile([40, 128], F32, tag="prep_in")
        nc.sync.dma_start(prep_in[0:32, :], mask_d.ap())
        nc.sync.dma_start(prep_in[32:40, :], scale_d.ap())
        # bias = (1-mask) * -1e4 = mask*1e4 - 1e4
        nc.vector.tensor_scalar(prep_in[0:32, :], prep_in[0:32, :],
                                1.0e4, -1.0e4, ALU.mult, ALU.add)
        tp = psa.tile([128, 40], F32, tag="prep_ps")
        nc.tensor.transpose(tp[:], prep_in[:], ident[0:40, 0:40])
        nc.vector.tensor_copy(prep_T[:], tp[:])

        # weights: load, scale-fold (wq), round to f32r
        wst = sa.tile([128, NKT, CS], F32, tag="wstage")
        nc.sync.dma_start(wst[:], wq_d.ap().rearrange("(k p) c -> p k c", p=128))
        for kt in range(NKT):
            nc.vector.tensor_scalar_mul(wq_r[:, kt, :], wst[:, kt, :],
                                        prep_T[:, 32 + kt:33 + kt])
        wst2 = sa.tile([128, NKT, CS], F32, tag="wstage")
        nc.sync.dma_start(wst2[:], wk_d.ap().rearrange("(k p) c -> p k c", p=128))
        nc.vector.tensor_copy(wk_r[:], wst2[:])
        wst3 = sa.tile([128, NKT, CS], F32, tag="wstage")
        nc.sync.dma_start(wst3[:], wv_d.ap().rearrange("(k p) c -> p k c", p=128))
        nc.vector.tensor_copy(wv_r[:], wst3[:])
        # v ones columns
        for c in range(NCH):
            nc.vector.memset(
                v_sb[c][:].rearrange("p s (h e) -> p s h e", h=HPC)[:, :, :, 64:65],
                1.0)

    # ---------------- stage B: x-path (norm, transpose, q) ----------------
    with (
        tc.tile_pool(name="stageB", bufs=1) as sb_,
        tc.tile_pool(name="stageB2", bufs=2) as sb2,
        tc.tile_pool(name="psB", bufs=2, space="PSUM") as psb,
    ):
        x_full = sb_.tile([128, NLT, D], F32, tag="x_full")
        x_view = x_d.ap().rearrange("(k p) c -> p k c", p=128)
        for lt in range(NLT):
            nc.sync.dma_start(x_full[:, lt, :], x_view[:, lt, :])
        ss = sb_.tile([128, NLT], F32, tag="ss")
        for lt in range(NLT):
            sq_scratch = sb2.tile([128, D], BF16, tag="sq")
            nc.scalar.activation(sq_scratch[:], x_full[:, lt, :], AF.Square,
                                 accum_out=ss[:, lt:lt + 1])
        # norm chain: r=sqrt(ss); newton; inv = 1/(r/32 + eps)
        nrm = sb_.tile([128, NLT * 4], F32, tag="nrm")
        r0 = nrm[:, 0:NLT]
        inv_r = nrm[:, NLT:2 * NLT]
        t2 = nrm[:, 2 * NLT:3 * NLT]
        inv_den = nrm[:, 3 * NLT:4 * NLT]
        nc.scalar.sqrt(r0, ss[:])
        nc.vector.reciprocal(inv_r, r0)
        nc.vector.tensor_mul(t2, ss[:], inv_r)
        nc.vector.tensor_add(t2, t2, r0)          # t2 = r + ss/r = 2*sqrt(ss) refined
        nc.vector.tensor_scalar(t2, t2, 0.5 / 32.0, 1.0e-8, ALU.mult, ALU.add)
        nc.vector.reciprocal(inv_den, t2)

        xnT = sb_.tile([128, NKT, L], F32R, tag="xnT")
        for lt in range(NLT):
            xn = sb2.tile([128, D], F32R, tag="xn")
            nc.vector.tensor_scalar_mul(xn[:], x_full[:, lt, :],
                                        inv_den[:, lt:lt + 1])
            for grp in range(2):
                tp = psb.tile([128, 512], F32R, tag="tpB")
                for j in range(4):
                    kb = grp * 4 + j
                    nc.tensor.transpose(tp[:, j * 128:(j + 1) * 128],
                                        xn[:, kb * 128:(kb + 1) * 128], ident_r[:])
                nc.vector.tensor_copy(
                    xnT[:, grp * 4:(grp + 1) * 4, lt * 128:(lt + 1) * 128],
                    tp[:].rearrange("p (a b) -> p a b", a=4))
        if "xnT" in dbg:
            nc.sync.dma_start(dbg["xnT"].ap(), xnT[:])
        # q projection: qT[pair, l] = Wq^T @ xnT
        for m in range(2):
            for lcq in range(2):
                qp = psb.tile([128, 512], F32, tag="qps")
                for kt in range(NKT):
                    nc.tensor.matmul(qp[:],
                                     wq_r[:, kt, m * 128:(m + 1) * 128],
                                     xnT[:, kt, lcq * 512:(lcq + 1) * 512],
                                     start=(kt == 0), stop=(kt == NKT - 1))
                nc.vector.tensor_copy(qT[:, m, lcq * 512:(lcq + 1) * 512], qp[:])

    # ------------- stages C+D: kv projection interleaved with attention -------------
    with (
        tc.tile_pool(name="stageC", bufs=2) as sc_,
        tc.tile_pool(name="stageD", bufs=1) as sd_,
        tc.tile_pool(name="probs", bufs=2) as sdp,
        tc.tile_pool(name="dsmall", bufs=2) as sds,
        tc.tile_pool(name="psC", bufs=2, space="PSUM") as psc,
        tc.tile_pool(name="psSC", bufs=2, space="PSUM") as ps_sc,
        tc.tile_pool(name="psAT", bufs=2, space="PSUM") as ps_at,
    ):
        for ch in range(NCH):
            canat = sc_.tile([128, SCH // 128, CA], F32R, tag="canat")
            nc.gpsimd.dma_start(
                canat[:],
                ca_d.ap().rearrange("(c t p) d -> c p t d", p=128, t=SCH // 128)[ch])
            caT = sc_.tile([128, NKT, SCH], F32R, tag="caT")
            for pt in range(SCH // 128):
                for grp in range(2):
                    tp = psc.tile([128, 512], F32R, tag="cps", name="tp")
                    for j in range(4):
                        kb = grp * 4 + j
                        nc.tensor.transpose(tp[:, j * 128:(j + 1) * 128],
                                            canat[:, pt, kb * 128:(kb + 1) * 128],
                                            ident_r[:])
                    nc.vector.tensor_copy(
                        caT[:, grp * 4:(grp + 1) * 4, pt * 128:(pt + 1) * 128],
                        tp[:].rearrange("p (a b) -> p a b", a=4))
            # kT for this chunk
            for m in range(2):
                kp = psc.tile([128, SCH], F32, tag="cps", name="kp")
                for kt in range(NKT):
                    nc.tensor.matmul(kp[:],
                                     wk_r[:, kt, m * 128:(m + 1) * 128],
                                     caT[:, kt, :],
                                     start=(kt == 0), stop=(kt == NKT - 1))
                nc.vector.tensor_copy(kT[ch][:, m, :], kp[:])
            # v natural, with M=65 interleave layout
            for st in range(SCH // 128):
                vp = psc.tile([128, CS], F32, tag="cps", name="vp")
                for kt in range(NKT):
                    nc.tensor.matmul(vp[:],
                                     caT[:, kt, st * 128:(st + 1) * 128],
                                     wv_r[:, kt, :],
                                     start=(kt == 0), stop=(kt == NKT - 1))
                nc.vector.tensor_copy(
                    v_sb[ch][:, st, :].rearrange("p (h e) -> p h e", h=HPC)[:, :, 0:64],
                    vp[:].rearrange("p (h e) -> p h e", h=HPC))

        # ---------------- attention ----------------
        # wo load here: DMA engines are past the input-load burst by now
        nc.gpsimd.dma_start(wo_bf[:], wo_d.ap().rearrange("(k p) c -> p k c", p=128))
        attn_n = [sd_.tile([64, L], BF16, tag=f"attn_n{h}", name=f"attn_n{h}")
                  for h in range(HPC)]
        TPC = SCH // 128  # s-tiles per chunk
        for lc in range(2):
            for hp in range(2):
                at = [ps_at.tile([65, 512], F32, tag="attn", name=f"at{i}")
                      for i in range(2)]
                for t in range(NST):
                    scps = ps_sc.tile([128, 1024], F32, tag="sc")
                    for i in range(2):
                        nc.tensor.matmul(
                            scps[:, i * 512:(i + 1) * 512],
                            kT[t // TPC][i * 64:(i + 1) * 64, hp,
                                         (t % TPC) * 128:(t % TPC + 1) * 128],
                            qT[i * 64:(i + 1) * 64, hp, lc * 512:(lc + 1) * 512],
                            start=True, stop=True)
                    probs = sdp.tile([128, 1024], BF16, tag="probs", bufs=3)
                    nc.scalar.activation(probs[:], scps[:], AF.Exp,
                                         bias=prep_T[:, t:t + 1], scale=1.0)
                    if lc == 0 and hp == 0 and t == 0 and "probs" in dbg:
                        nc.sync.dma_start(dbg["probs"].ap(), probs[:])
                    for i in range(2):
                        h = hp * 2 + i
                        nc.tensor.matmul(
                            at[i],
                            v_sb[t // TPC][:, t % TPC, h * 65:(h + 1) * 65],
                            probs[:, i * 512:(i + 1) * 512],
                            start=(t == 0), stop=(t == NST - 1))
                # copy raw accumulators out of PSUM immediately so the
                # next pair's PV can start; normalize from SBUF afterwards
                raw = [sds.tile([65, 512], F32, tag="rawat", bufs=4,
                                name=f"raw{i}") for i in range(2)]
                for i in range(2):
                    nc.vector.tensor_copy(raw[i][:], at[i][:])
                dsb = sds.tile([128, 1024], F32, tag="dsb", bufs=1)
                rsb = sds.tile([128, 1024], F32R, tag="rsb", bufs=1)
                for i in range(2):
                    nc.vector.reciprocal(dsb[64:65, i * 512:(i + 1) * 512],
                                         raw[i][64:65, :])
                nc.vector.tensor_copy(rsb[64:65, :], dsb[64:65, :])
                if lc == 0 and hp == 0:
                    if "dsb" in dbg:
                        nc.sync.dma_start(dbg["dsb"].ap(), dsb[:])
                    if "rsb" in dbg:
                        nc.sync.dma_start(dbg["rsb"].ap(), rsb[:])
                for i in range(2):
                    h = hp * 2 + i
                    dbc = ps_sc.tile([64, 512], F32, tag="sc", name="dbc")
                    nc.tensor.matmul(dbc[:], ones_r[64:65, 0:64],
                                     rsb[64:65, i * 512:(i + 1) * 512],
                                     tile_position=(64, 0),
                                     start=True, stop=True)
                    dbc_sb = sds.tile([64, 512], F32, tag="dbc_sb")
                    nc.vector.tensor_copy(dbc_sb[:], dbc[:])
                    if lc == 0 and hp == 0 and i == 0 and "dbc" in dbg:
                        nc.sync.dma_start(dbg["dbc"].ap(), dbc_sb[:])
                    nc.vector.tensor_mul(
                        attn_n[h][:, lc * 512:(lc + 1) * 512],
                        raw[i][0:64, :], dbc_sb[:])
        # A2A staging: shard j goes to core j; shards for the other batch's
        # cores are zeroed via bsel so the receiver can just sum both halves.
        for h in range(HPC):
            a2a_st = sds.tile([64, N_CORES, LSL], BF16, tag="a2a_st", bufs=1,
                              name=f"a2a_st{h}")
            for half in range(2):
                nc.vector.tensor_scalar_mul(
                    a2a_st[:, half * G:(half + 1) * G, :],
                    attn_n[h][:].rearrange("p (j l) -> p j l", j=G),
                    bsel_sb[:, half:half + 1])
            nc.gpsimd.dma_start(
                a2a_in[:, h * 64:(h + 1) * 64, :].rearrange("j p l -> p j l"),
                a2a_st[:])
        if "attn0" in dbg:
            nc.sync.dma_start(dbg["attn0"].ap(), attn_n[0][:])
        if "qT" in dbg:
            nc.sync.dma_start(dbg["qT"].ap(), qT[:])

    nc.gpsimd.collective_compute(
        "AllToAll", ALU.bypass,
        replica_groups=[list(range(N_CORES))],
        ins=[a2a_in.opt()], outs=[a2a_out.opt()])

    # ---------------- stage E: final projection ----------------
    with (
        tc.tile_pool(name="stageE", bufs=1) as se_,
        tc.tile_pool(name="stageE2", bufs=2) as se2,
        tc.tile_pool(name="psE", bufs=2, space="PSUM") as pse,
    ):
        attnTa = se_.tile([128, NKT, LSL], BF16, tag="attnTa")
        attnTb = se_.tile([128, NKT, LSL], BF16, tag="attnTb")
        nc.gpsimd.dma_start(
            attnTa[:],
            a2a_out[0:G].rearrange("i (t p) l -> p (i t) l", p=128))
        nc.gpsimd.dma_start(
            attnTb[:],
            a2a_out[G:2 * G].rearrange("i (t p) l -> p (i t) l", p=128))
        attnT = se_.tile([128, NKT, LSL], BF16, tag="attnT")
        nc.vector.tensor_add(attnT[:], attnTa[:], attnTb[:])
        if "a2a" in dbg:
            nc.sync.dma_start(dbg["a2a"].ap(), a2a_out[:])
        for mt in range(2):
            ysb = se2.tile([128, D], F32, tag="ysb")
            for en in range(2):
                yp = pse.tile([128, 512], F32, tag="yps")
                for kt in range(NKT):
                    nc.tensor.matmul(yp[:],
                                     attnT[:, kt, mt * 128:(mt + 1) * 128],
                                     wo_bf[:, kt, en * 512:(en + 1) * 512],
                                     start=(kt == 0), stop=(kt == NKT - 1))
                nc.vector.tensor_copy(ysb[:, en * 512:(en + 1) * 512], yp[:])
            nc.gpsimd.dma_start(y_d.ap()[mt * 128:(mt + 1) * 128, :], ysb[:])


_NC_CACHE = {}


def _get_nc(repeat=1):
    if repeat not in _NC_CACHE:
        _NC_CACHE[repeat] = build_program(repeat)
    return _NC_CACHE[repeat]


def make_in_maps(inputs):
    x = np.ascontiguousarray(np.asarray(inputs["hidden_states"], dtype=np.float32))
    ca = np.ascontiguousarray(np.asarray(inputs["ca_hidden_states"], dtype=np.float32))
    mask = np.asarray(inputs["ca_attention_mask"], dtype=np.float32)
    scale = np.asarray(inputs["scale"], dtype=np.float32)
    Wq = np.asarray(inputs["Wq"], dtype=np.float32)
    Wkv = np.asarray(inputs["Wkv"], dtype=np.float32)
    Wo = np.ascontiguousarray(np.asarray(inputs["Wo"], dtype=np.float32))
    in_maps = []
    for c in range(N_CORES):
        b, g = c // G, c % G
        bsel = np.zeros((64, 2), np.float32)
        bsel[:, b] = 1.0
        in_maps.append({
            "bsel": bsel,
            "x": x[b],
            "ca": ca[b],
            "mask": np.ascontiguousarray(mask[b].reshape(32, 128)),
            "scale": np.ascontiguousarray(scale.reshape(NKT, 128)),
            "wq": np.ascontiguousarray(Wq[:, g * CS:(g + 1) * CS]),
            "wk": np.ascontiguousarray(Wkv[:, g * CS:(g + 1) * CS]),
            "wv": np.ascontiguousarray(Wkv[:, CA + g * CS:CA + (g + 1) * CS]),
            "wo": Wo,
        })
    return in_maps


def kernel(**inputs) -> np.ndarray:
    nc = _get_nc(1)
    in_maps = make_in_maps(inputs)
    res = run_bass_kernel_spmd(nc, in_maps, core_ids=list(range(N_CORES)))
    out = np.empty((B, L, D), dtype=np.float32)
    for c in range(N_CORES):
        b, g = c // G, c % G
        out[b, g * LSL:(g + 1) * LSL, :] = res.results[c]["y"]
    return out


# revision 41
# speedup vs baseline: 1.0019x; 1.0019x over previous
"""TRN2 Bass kernel for nn_ActionDecoderCrossAttention.

Sharding: 8 cores = 2 (batch) x 4 (head-groups of 4 heads).
Per core: RMSNorm(x) -> qT (f32r), caT via PE-transpose (f32r),
kT/v projections, scoresT = k @ qT per head (f32r), exp on ACT with mask bias
(no max-subtraction: |score| < 70 by construction), PV with an appended
ones-column (M=65) so softmax denominators accumulate in the same matmul,
normalize, 8-way AllToAll (cross-batch shards zeroed via the per-core bsel
input) to redistribute attn^T by L-slice, final projection vs full Wo (bf16),
output y slice [256, 1024] fp32.

All stage pools coexist in SBUF so kv projection, attention, and input loads
pipeline freely; only stage E aliases stage B memory (benign WAR).

Self-contained: hardcodes all shapes; builds the Bass program once per process.
"""

import numpy as np

import concourse.bass as bass
import concourse.mybir as mybir
import concourse.tile as tile
from concourse import bacc
from concourse.bass_utils import run_bass_kernel_spmd
from concourse.masks import make_identity

F32 = mybir.dt.float32
F32R = mybir.dt.float32r
BF16 = mybir.dt.bfloat16
AF = mybir.ActivationFunctionType
ALU = mybir.AluOpType

B, L, S, D, CA, H, DH = 2, 1024, 4096, 1024, 1024, 16, 64
N_CORES = 8
G = 4               # head-groups (tensor-parallel degree per batch)
HPC = H // G        # heads per core = 4
CS = HPC * DH       # per-core CA shard = 256
LSL = L // G        # output L-slice per core = 256
NKT = D // 128      # 8 contraction tiles
NLT = L // 128      # 8 L tiles
NST = S // 128      # 32 S tiles
SCH = 256           # ca processing chunk (s dimension)
NCH = S // SCH      # 16 chunks
TPC = SCH // 128    # s-tiles per chunk = 2


def build_program(repeat=1, debug=False):
    nc = bacc.Bacc("TRN2", target_bir_lowering=False, debug=False,
                   num_devices=N_CORES)
    dbg = {}
    if debug:
        dbg["qT"] = nc.dram_tensor("dbg_qT", [128, 2, L], F32R, kind="ExternalOutput")
        dbg["probs"] = nc.dram_tensor("dbg_probs", [128, 1024], BF16, kind="ExternalOutput")
        dbg["attn0"] = nc.dram_tensor("dbg_attn0", [64, L], BF16, kind="ExternalOutput")
        dbg["a2a"] = nc.dram_tensor("dbg_a2a", [N_CORES, CS, LSL], BF16, kind="ExternalOutput")

    bsel_d = nc.dram_tensor("bsel", [64, 2], F32, kind="ExternalInput")
    x_d = nc.dram_tensor("x", [L, D], F32, kind="ExternalInput")
    ca_d = nc.dram_tensor("ca", [S, CA], F32, kind="ExternalInput")
    mask_d = nc.dram_tensor("mask", [32, 128], F32, kind="ExternalInput")
    scale_d = nc.dram_tensor("scale", [NKT, 128], F32, kind="ExternalInput")
    wq_d = nc.dram_tensor("wq", [D, CS], F32, kind="ExternalInput")
    wk_d = nc.dram_tensor("wk", [CA, CS], F32, kind="ExternalInput")
    wv_d = nc.dram_tensor("wv", [CA, CS], F32, kind="ExternalInput")
    wo_d = nc.dram_tensor("wo", [CA, D], F32, kind="ExternalInput")
    y_d = nc.dram_tensor("y", [LSL, D], F32, kind="ExternalOutput")

    with tile.TileContext(nc) as tc:
        with (
            tc.tile_pool(name="persist", bufs=1) as pp,
            tc.tile_pool(name="dram", bufs=1, space="DRAM") as dp,
        ):
            a2a_in = dp.tile([N_CORES, CS, LSL], BF16, tag="a2a_in")
            a2a_out = dp.tile([N_CORES, CS, LSL], BF16, tag="a2a_out")
            bsel_sb = pp.tile([64, 2], F32, tag="bsel_sb")
            nc.sync.dma_start(bsel_sb[:], bsel_d.ap())
            ident = pp.tile([128, 128], F32, tag="ident")
            make_identity(nc, ident[:])
            ident_r = pp.tile([128, 128], F32R, tag="ident_r")
            nc.vector.tensor_copy(ident_r[:], ident[:])
            prep_T = pp.tile([128, 40], F32, tag="prepT")  # 0-31 exp bias, 32-39 scale
            wq_r = pp.tile([128, NKT, CS], F32R, tag="wq_r")
            wk_r = pp.tile([128, NKT, CS], F32R, tag="wk_r")
            wv_r = pp.tile([128, NKT, CS], F32R, tag="wv_r")
            wo_bf = pp.tile([128, NKT, D], BF16, tag="wo_bf")
            qT = pp.tile([128, 2, L], F32R, tag="qT")
            kT = [pp.tile([128, 2, SCH], F32R, tag=f"kT{c}", name=f"kT{c}")
                  for c in range(NCH)]
            v_sb = [pp.tile([128, TPC, HPC * 65], BF16, tag=f"v_sb{c}",
                            name=f"v_sb{c}") for c in range(NCH)]
            ones_r = pp.tile([128, 64], F32R, tag="ones_r")
            ones_f = pp.tile([128, 64], F32, tag="ones_f")
            nc.vector.memset(ones_f[:], 1.0)
            nc.vector.tensor_copy(ones_r[:], ones_f[:])

            for _rep in range(repeat):
                _emit_body(nc, tc, pp, ident, ident_r, prep_T, wq_r, wk_r, wv_r,
                           wo_bf, qT, kT, v_sb, ones_r, bsel_sb,
                           x_d, ca_d, mask_d, scale_d, wq_d, wk_d, wv_d, wo_d,
                           y_d, a2a_in, a2a_out, dbg)

    nc.finalize()
    return nc


def _emit_body(nc, tc, pp, ident, ident_r, prep_T, wq_r, wk_r, wv_r, wo_bf,
               qT, kT, v_sb, ones_r, bsel_sb,
               x_d, ca_d, mask_d, scale_d, wq_d, wk_d, wv_d, wo_d,
               y_d, a2a_in, a2a_out, dbg=None):
    dbg = dbg or {}
    # ---------------- stage A: prep (mask bias, scale, weights) ----------------
    with (
        tc.tile_pool(name="stageA", bufs=2) as sa,
        tc.tile_pool(name="psA", bufs=2, space="PSUM") as psa,
    ):
        prep_in = sa.tile([40, 128], F32, tag="prep_in")
        nc.sync.dma_start(prep_in[0:32, :], mask_d.ap())
        nc.sync.dma_start(prep_in[32:40, :], scale_d.ap())
        # bias = (1-mask) * -1e4 = mask*1e4 - 1e4
        nc.vector.tensor_scalar(prep_in[0:32, :], prep_in[0:32, :],
                                1.0e4, -1.0e4, ALU.mult, ALU.add)
        tp0 = psa.tile([128, 40], F32, tag="prep_ps")
        nc.tensor.transpose(tp0[:], prep_in[:], ident[0:40, 0:40])
        nc.vector.tensor_copy(prep_T[:], tp0[:])

        # weights: load, scale-fold (wq), round to f32r
        wst = sa.tile([128, NKT, CS], F32, tag="wstage")
        nc.sync.dma_start(wst[:], wq_d.ap().rearrange("(k p) c -> p k c", p=128))
        for kt in range(NKT):
            nc.vector.tensor_scalar_mul(wq_r[:, kt, :], wst[:, kt, :],
                                        prep_T[:, 32 + kt:33 + kt])
        wst2 = sa.tile([128, NKT, CS], F32, tag="wstage")
        nc.sync.dma_start(wst2[:], wk_d.ap().rearrange("(k p) c -> p k c", p=128))
        nc.vector.tensor_copy(wk_r[:], wst2[:])
        wst3 = sa.tile([128, NKT, CS], F32, tag="wstage")
        nc.sync.dma_start(wst3[:], wv_d.ap().rearrange("(k p) c -> p k c", p=128))
        nc.vector.tensor_copy(wv_r[:], wst3[:])
        # v ones columns
        for c in range(NCH):
            nc.vector.memset(
                v_sb[c][:].rearrange("p s (h e) -> p s h e", h=HPC)[:, :, :, 64:65],
                1.0)

    # ------- stages B+C+D: all pools coexist; Tile pipelines across them -------
    with (
        tc.tile_pool(name="stageB", bufs=1) as sb_,
        tc.tile_pool(name="stageB2", bufs=2) as sb2,
        tc.tile_pool(name="stageC", bufs=2) as sc_,
        tc.tile_pool(name="stageD", bufs=1) as sd_,
        tc.tile_pool(name="probs", bufs=3) as sdp,
        tc.tile_pool(name="dsmall", bufs=2) as sds,
        tc.tile_pool(name="psC", bufs=2, space="PSUM") as psc,
        tc.tile_pool(name="psSC", bufs=2, space="PSUM") as ps_sc,
        tc.tile_pool(name="psAT", bufs=2, space="PSUM") as ps_at,
    ):
        # ---------------- stage B: x-path (norm, transpose, q) ----------------
        x_view = x_d.ap().rearrange("(k p) c -> p k c", p=128)
        xnT = sb_.tile([128, NKT, 512], F32R, tag="xnT")
        ss = sb_.tile([128, NLT], F32, tag="ss")
        nrm = sb_.tile([128, NLT * 4], F32, tag="nrm")
        for lh in range(2):
            for j in range(4):
                lt = lh * 4 + j
                x_t = sb2.tile([128, D], F32, tag="x_t")
                nc.sync.dma_start(x_t[:], x_view[:, lt, :])
                sq_scratch = sb2.tile([128, D], BF16, tag="sq", bufs=1)
                nc.scalar.activation(sq_scratch[:], x_t[:], AF.Square,
                                     accum_out=ss[:, lt:lt + 1])
                r0 = nrm[:, 4 * lt + 0:4 * lt + 1]
                inv_r = nrm[:, 4 * lt + 1:4 * lt + 2]
                t2 = nrm[:, 4 * lt + 2:4 * lt + 3]
                inv_den = nrm[:, 4 * lt + 3:4 * lt + 4]
                nc.scalar.sqrt(r0, ss[:, lt:lt + 1])
                nc.vector.reciprocal(inv_r, r0)
                nc.vector.tensor_mul(t2, ss[:, lt:lt + 1], inv_r)
                nc.vector.tensor_add(t2, t2, r0)
                nc.vector.tensor_scalar(t2, t2, 0.5 / 32.0, 1.0e-8,
                                        ALU.mult, ALU.add)
                nc.vector.reciprocal(inv_den, t2)
                xn = sb2.tile([128, D], F32R, tag="xn")
                nc.vector.tensor_scalar_mul(xn[:], x_t[:], inv_den)
                for grp in range(2):
                    tp = psc.tile([128, 512], F32R, tag="cps", name="tpB")
                    for jj in range(4):
                        kb = grp * 4 + jj
                        nc.tensor.transpose(tp[:, jj * 128:(jj + 1) * 128],
                                            xn[:, kb * 128:(kb + 1) * 128],
                                            ident_r[:])
                    nc.vector.tensor_copy(
                        xnT[:, grp * 4:(grp + 1) * 4, j * 128:(j + 1) * 128],
                        tp[:].rearrange("p (a b) -> p a b", a=4))
            for m in range(2):
                qp = psc.tile([128, 512], F32, tag="cps", name="qps")
                for kt in range(NKT):
                    nc.tensor.matmul(qp[:],
                                     wq_r[:, kt, m * 128:(m + 1) * 128],
                                     xnT[:, kt, :],
                                     start=(kt == 0), stop=(kt == NKT - 1))
                nc.vector.tensor_copy(qT[:, m, lh * 512:(lh + 1) * 512], qp[:])
        if "qT" in dbg:
            nc.sync.dma_start(dbg["qT"].ap(), qT[:])

        # ---------------- stage C: ca transpose + kv projections ----------------
        ca_view = ca_d.ap().rearrange("(c t p) d -> c p t d", p=128, t=TPC)
        for ch in range(NCH):
            canat = sc_.tile([128, TPC, CA], F32R, tag="canat")
            nc.gpsimd.dma_start(canat[:], ca_view[ch])
            caT = sc_.tile([128, NKT, SCH], F32R, tag="caT")
            for pt in range(TPC):
                for grp in range(2):
                    tp = psc.tile([128, 512], F32R, tag="cps", name="tp")
                    for jj in range(4):
                        kb = grp * 4 + jj
                        nc.tensor.transpose(tp[:, jj * 128:(jj + 1) * 128],
                                            canat[:, pt, kb * 128:(kb + 1) * 128],
                                            ident_r[:])
                    nc.vector.tensor_copy(
                        caT[:, grp * 4:(grp + 1) * 4, pt * 128:(pt + 1) * 128],
                        tp[:].rearrange("p (a b) -> p a b", a=4))
            for m in range(2):
                kp = psc.tile([128, SCH], F32, tag="cps", name="kp")
                for kt in range(NKT):
                    nc.tensor.matmul(kp[:],
                                     wk_r[:, kt, m * 128:(m + 1) * 128],
                                     caT[:, kt, :],
                                     start=(kt == 0), stop=(kt == NKT - 1))
                nc.vector.tensor_copy(kT[ch][:, m, :], kp[:])
            for st in range(TPC):
                vp = psc.tile([128, CS], F32, tag="cps", name="vp")
                for kt in range(NKT):
                    nc.tensor.matmul(vp[:],
                                     caT[:, kt, st * 128:(st + 1) * 128],
                                     wv_r[:, kt, :],
                                     start=(kt == 0), stop=(kt == NKT - 1))
                nc.vector.tensor_copy(
                    v_sb[ch][:, st, :].rearrange("p (h e) -> p h e", h=HPC)[:, :, 0:64],
                    vp[:].rearrange("p (h e) -> p h e", h=HPC))

        # ---------------- stage D: attention ----------------
        # wo load here: DMA engines are past the input-load burst by now
        nc.gpsimd.dma_start(wo_bf[:], wo_d.ap().rearrange("(k p) c -> p k c", p=128))
        attn_n = [sd_.tile([64, L], BF16, tag=f"attn_n{h}", name=f"attn_n{h}")
                  for h in range(HPC)]
        for lc in range(2):
            for hp in range(2):
                at = [ps_at.tile([65, 512], F32, tag="attn", name=f"at{i}")
                      for i in range(2)]
                for t in range(NST):
                    scps = ps_sc.tile([128, 1024], F32, tag="sc")
                    for i in range(2):
                        nc.tensor.matmul(
                            scps[:, i * 512:(i + 1) * 512],
                            kT[t // TPC][i * 64:(i + 1) * 64, hp,
                                         (t % TPC) * 128:(t % TPC + 1) * 128],
                            qT[i * 64:(i + 1) * 64, hp, lc * 512:(lc + 1) * 512],
                            start=True, stop=True)
                    probs = sdp.tile([128, 1024], BF16, tag="probs", bufs=3)
                    nc.scalar.activation(probs[:], scps[:], AF.Exp,
                                         bias=prep_T[:, t:t + 1], scale=1.0)
                    if lc == 0 and hp == 0 and t == 0 and "probs" in dbg:
                        nc.sync.dma_start(dbg["probs"].ap(), probs[:])
                    for i in range(2):
                        h = hp * 2 + i
                        nc.tensor.matmul(
                            at[i],
                            v_sb[t // TPC][:, t % TPC, h * 65:(h + 1) * 65],
                            probs[:, i * 512:(i + 1) * 512],
                            start=(t == 0), stop=(t == NST - 1))
                # copy raw accumulators out of PSUM immediately; normalize
                # from SBUF off the critical path
                raw = [sds.tile([65, 512], F32, tag="rawat", bufs=4,
                                name=f"raw{i}") for i in range(2)]
                for i in range(2):
                    nc.vector.tensor_copy(raw[i][:], at[i][:])
                dinv = sds.tile([128, 1024], F32R, tag="dinv", bufs=1)
                with nc.allow_low_precision(reason="f32r rounding of 1/D is benign"):
                    for i in range(2):
                        nc.vector.reciprocal(dinv[64:65, i * 512:(i + 1) * 512],
                                             raw[i][64:65, :])
                for i in range(2):
                    h = hp * 2 + i
                    dbc = ps_sc.tile([64, 512], F32, tag="sc", name="dbc")
                    nc.tensor.matmul(dbc[:], ones_r[64:65, 0:64],
                                     dinv[64:65, i * 512:(i + 1) * 512],
                                     tile_position=(64, 0),
                                     start=True, stop=True)
                    dbc_sb = sds.tile([64, 512], F32, tag="dbc_sb")
                    nc.vector.tensor_copy(dbc_sb[:], dbc[:])
                    nc.vector.tensor_mul(
                        attn_n[h][:, lc * 512:(lc + 1) * 512],
                        raw[i][0:64, :], dbc_sb[:])
        # A2A staging: shard j goes to core j; the other batch's shards are
        # zeroed via bsel so the receiver can just sum both halves.
        for h in range(HPC):
            a2a_st = sds.tile([64, N_CORES, LSL], BF16, tag="a2a_st", bufs=1,
                              name=f"a2a_st{h}")
            for half in range(2):
                nc.vector.tensor_scalar_mul(
                    a2a_st[:, half * G:(half + 1) * G, :],
                    attn_n[h][:].rearrange("p (j l) -> p j l", j=G),
                    bsel_sb[:, half:half + 1])
            nc.gpsimd.dma_start(
                a2a_in[:, h * 64:(h + 1) * 64, :].rearrange("j p l -> p j l"),
                a2a_st[:])
        if "attn0" in dbg:
            nc.sync.dma_start(dbg["attn0"].ap(), attn_n[0][:])

    nc.gpsimd.collective_compute(
        "AllToAll", ALU.bypass,
        replica_groups=[list(range(N_CORES))],
        ins=[a2a_in.opt()], outs=[a2a_out.opt()])

    # ---------------- stage E: final projection ----------------
    with (
        tc.tile_pool(name="stageE", bufs=1) as se_,
        tc.tile_pool(name="stageE2", bufs=2) as se2,
        tc.tile_pool(name="psE", bufs=2, space="PSUM") as pse,
    ):
        attnTa = se_.tile([128, NKT, LSL], BF16, tag="attnTa")
        attnTb = se_.tile([128, NKT, LSL], BF16, tag="attnTb")
        nc.gpsimd.dma_start(
            attnTa[:],
            a2a_out[0:G].rearrange("i (t p) l -> p (i t) l", p=128))
        nc.gpsimd.dma_start(
            attnTb[:],
            a2a_out[G:2 * G].rearrange("i (t p) l -> p (i t) l", p=128))
        attnT = se_.tile([128, NKT, LSL], BF16, tag="attnT")
        nc.vector.tensor_add(attnT[:], attnTa[:], attnTb[:])
        if "a2a" in dbg:
            nc.sync.dma_start(dbg["a2a"].ap(), a2a_out[:])
        for mt in range(2):
            ysb = se2.tile([128, D], F32, tag="ysb")
            for en in range(2):
                yp = pse.tile([128, 512], F32, tag="yps")
                for kt in range(NKT):
                    nc.tensor.matmul(yp[:],
                                     attnT[:, kt, mt * 128:(mt + 1) * 128],
                                     wo_bf[:, kt, en * 512:(en + 1) * 512],
                                     start=(kt == 0), stop=(kt == NKT - 1))
                nc.vector.tensor_copy(ysb[:, en * 512:(en + 1) * 512], yp[:])
            nc.gpsimd.dma_start(y_d.ap()[mt * 128:(mt + 1) * 128, :], ysb[:])


_NC_CACHE = {}


def _get_nc(repeat=1):
    if repeat not in _NC_CACHE:
        _NC_CACHE[repeat] = build_program(repeat)
    return _NC_CACHE[repeat]


def make_in_maps(inputs):
    x = np.ascontiguousarray(np.asarray(inputs["hidden_states"], dtype=np.float32))
    ca = np.ascontiguousarray(np.asarray(inputs["ca_hidden_states"], dtype=np.float32))
    mask = np.asarray(inputs["ca_attention_mask"], dtype=np.float32)
    scale = np.asarray(inputs["scale"], dtype=np.float32)
    Wq = np.asarray(inputs["Wq"], dtype=np.float32)
    Wkv = np.asarray(inputs["Wkv"], dtype=np.float32)
    Wo = np.ascontiguousarray(np.asarray(inputs["Wo"], dtype=np.float32))
    in_maps = []
    for c in range(N_CORES):
        b, g = c // G, c % G
        bsel = np.zeros((64, 2), np.float32)
        bsel[:, b] = 1.0
        in_maps.append({
            "bsel": bsel,
            "x": x[b],
            "ca": ca[b],
            "mask": np.ascontiguousarray(mask[b].reshape(32, 128)),
            "scale": np.ascontiguousarray(scale.reshape(NKT, 128)),
            "wq": np.ascontiguousarray(Wq[:, g * CS:(g + 1) * CS]),
            "wk": np.ascontiguousarray(Wkv[:, g * CS:(g + 1) * CS]),
            "wv": np.ascontiguousarray(Wkv[:, CA + g * CS:CA + (g + 1) * CS]),
            "wo": Wo,
        })
    return in_maps


def kernel(**inputs) -> np.ndarray:
    nc = _get_nc(1)
    in_maps = make_in_maps(inputs)
    res = run_bass_kernel_spmd(nc, in_maps, core_ids=list(range(N_CORES)))
    out = np.empty((B, L, D), dtype=np.float32)
    for c in range(N_CORES):
        b, g = c // G, c % G
        out[b, g * LSL:(g + 1) * LSL, :] = res.results[c]["y"]
    return out


# revision 42
# speedup vs baseline: 25075.7754x; 25027.1436x over previous
"""TRN2 Bass kernel for nn_ActionDecoderCrossAttention.

Sharding: 8 cores = 2 (batch) x 4 (head-groups of 4 heads).
Per core: RMSNorm(x) -> qT (f32r), caT via PE-transpose (f32r),
kT/v projections, scoresT = k @ qT per head (f32r), exp on ACT with mask bias
(no max-subtraction: |score| < 70 by construction), PV with an appended
ones-column (M=65) so softmax denominators accumulate in the same matmul,
normalize, 8-way AllToAll (cross-batch shards zeroed via the per-core bsel
input) to redistribute attn^T by L-slice, final projection vs full Wo (bf16),
output y slice [256, 1024] fp32.

All stage pools coexist in SBUF so kv projection, attention, and input loads
pipeline freely; only stage E aliases stage B memory (benign WAR).

Self-contained: hardcodes all shapes; builds the Bass program once per process.
"""

import numpy as np

import concourse.bass as bass
import concourse.mybir as mybir
import concourse.tile as tile
from concourse import bacc
from concourse.bass_utils import run_bass_kernel_spmd
from concourse.masks import make_identity

F32 = mybir.dt.float32
F32R = mybir.dt.float32r
BF16 = mybir.dt.bfloat16
AF = mybir.ActivationFunctionType
ALU = mybir.AluOpType

B, L, S, D, CA, H, DH = 2, 1024, 4096, 1024, 1024, 16, 64
N_CORES = 8
G = 4               # head-groups (tensor-parallel degree per batch)
HPC = H // G        # heads per core = 4
CS = HPC * DH       # per-core CA shard = 256
LSL = L // G        # output L-slice per core = 256
NKT = D // 128      # 8 contraction tiles
NLT = L // 128      # 8 L tiles
NST = S // 128      # 32 S tiles
SCH = 256           # ca processing chunk (s dimension)
NCH = S // SCH      # 16 chunks
TPC = SCH // 128    # s-tiles per chunk = 2


def build_program(repeat=1, debug=False):
    nc = bacc.Bacc("TRN2", target_bir_lowering=False, debug=False,
                   num_devices=N_CORES)
    dbg = {}
    if debug:
        dbg["qT"] = nc.dram_tensor("dbg_qT", [128, 2, L], F32R, kind="ExternalOutput")
        dbg["probs"] = nc.dram_tensor("dbg_probs", [128, 1024], BF16, kind="ExternalOutput")
        dbg["attn0"] = nc.dram_tensor("dbg_attn0", [64, L], BF16, kind="ExternalOutput")
        dbg["a2a"] = nc.dram_tensor("dbg_a2a", [N_CORES, CS, LSL], BF16, kind="ExternalOutput")

    bsel_d = nc.dram_tensor("bsel", [64, 2], F32, kind="ExternalInput")
    x_d = nc.dram_tensor("x", [L, D], F32, kind="ExternalInput")
    ca_d = nc.dram_tensor("ca", [S, CA], F32, kind="ExternalInput")
    mask_d = nc.dram_tensor("mask", [32, 128], F32, kind="ExternalInput")
    scale_d = nc.dram_tensor("scale", [NKT, 128], F32, kind="ExternalInput")
    wq_d = nc.dram_tensor("wq", [D, CS], F32, kind="ExternalInput")
    wk_d = nc.dram_tensor("wk", [CA, CS], F32, kind="ExternalInput")
    wv_d = nc.dram_tensor("wv", [CA, CS], F32, kind="ExternalInput")
    wo_d = nc.dram_tensor("wo", [CA, D], F32, kind="ExternalInput")
    y_d = nc.dram_tensor("y", [LSL, D], F32, kind="ExternalOutput")

    with tile.TileContext(nc) as tc:
        with (
            tc.tile_pool(name="persist", bufs=1) as pp,
            tc.tile_pool(name="dram", bufs=1, space="DRAM") as dp,
        ):
            a2a_in = dp.tile([N_CORES, CS, LSL], BF16, tag="a2a_in")
            a2a_out = dp.tile([N_CORES, CS, LSL], BF16, tag="a2a_out")
            bsel_sb = pp.tile([64, 2], F32, tag="bsel_sb")
            nc.sync.dma_start(bsel_sb[:], bsel_d.ap())
            ident = pp.tile([128, 128], F32, tag="ident")
            make_identity(nc, ident[:])
            ident_r = pp.tile([128, 128], F32R, tag="ident_r")
            nc.vector.tensor_copy(ident_r[:], ident[:])
            prep_T = pp.tile([128, 40], F32, tag="prepT")  # 0-31 exp bias, 32-39 scale
            wq_r = pp.tile([128, NKT, CS], F32R, tag="wq_r")
            wk_r = pp.tile([128, NKT, CS], F32R, tag="wk_r")
            wv_r = pp.tile([128, NKT, CS], F32R, tag="wv_r")
            wo_bf = pp.tile([128, NKT, D], BF16, tag="wo_bf")
            qT = pp.tile([128, 2, L], F32R, tag="qT")
            kT = [pp.tile([128, 2, SCH], F32R, tag=f"kT{c}", name=f"kT{c}")
                  for c in range(NCH)]
            v_sb = [pp.tile([128, TPC, HPC * 65], BF16, tag=f"v_sb{c}",
                            name=f"v_sb{c}") for c in range(NCH)]
            ones_r = pp.tile([128, 64], F32R, tag="ones_r")
            ones_f = pp.tile([128, 64], F32, tag="ones_f")
            nc.vector.memset(ones_f[:], 1.0)
            nc.vector.tensor_copy(ones_r[:], ones_f[:])

            for _rep in range(repeat):
                _emit_body(nc, tc, pp, ident, ident_r, prep_T, wq_r, wk_r, wv_r,
                           wo_bf, qT, kT, v_sb, ones_r, bsel_sb,
                           x_d, ca_d, mask_d, scale_d, wq_d, wk_d, wv_d, wo_d,
                           y_d, a2a_in, a2a_out, dbg)

    nc.finalize()
    return nc


def _emit_body(nc, tc, pp, ident, ident_r, prep_T, wq_r, wk_r, wv_r, wo_bf,
               qT, kT, v_sb, ones_r, bsel_sb,
               x_d, ca_d, mask_d, scale_d, wq_d, wk_d, wv_d, wo_d,
               y_d, a2a_in, a2a_out, dbg=None):
    dbg = dbg or {}
    # ---------------- stage A: prep (mask bias, scale, weights) ----------------
    with (
        tc.tile_pool(name="stageA", bufs=2) as sa,
        tc.tile_pool(name="psA", bufs=2, space="PSUM") as psa,
    ):
        prep_in = sa.tile([40, 128], F32, tag="prep_in")
        nc.sync.dma_start(prep_in[0:32, :], mask_d.ap())
        nc.sync.dma_start(prep_in[32:40, :], scale_d.ap())
        # bias = (1-mask) * -1e4 = mask*1e4 - 1e4
        nc.vector.tensor_scalar(prep_in[0:32, :], prep_in[0:32, :],
                                1.0e4, -1.0e4, ALU.mult, ALU.add)
        tp0 = psa.tile([128, 40], F32, tag="prep_ps")
        nc.tensor.transpose(tp0[:], prep_in[:], ident[0:40, 0:40])
        nc.vector.tensor_copy(prep_T[:], tp0[:])

        # weights: load, scale-fold (wq), round to f32r
        wst = sa.tile([128, NKT, CS], F32, tag="wstage")
        nc.sync.dma_start(wst[:], wq_d.ap().rearrange("(k p) c -> p k c", p=128))
        for kt in range(NKT):
            nc.vector.tensor_scalar_mul(wq_r[:, kt, :], wst[:, kt, :],
                                        prep_T[:, 32 + kt:33 + kt])
        wst2 = sa.tile([128, NKT, CS], F32, tag="wstage")
        nc.sync.dma_start(wst2[:], wk_d.ap().rearrange("(k p) c -> p k c", p=128))
        nc.vector.tensor_copy(wk_r[:], wst2[:])
        wst3 = sa.tile([128, NKT, CS], F32, tag="wstage")
        nc.sync.dma_start(wst3[:], wv_d.ap().rearrange("(k p) c -> p k c", p=128))
        nc.vector.tensor_copy(wv_r[:], wst3[:])
        # v ones columns
        for c in range(NCH):
            nc.vector.memset(
                v_sb[c][:].rearrange("p s (h e) -> p s h e", h=HPC)[:, :, :, 64:65],
                1.0)

    # ------- stages B+C+D: all pools coexist; Tile pipelines across them -------
    with (
        tc.tile_pool(name="stageB", bufs=1) as sb_,
        tc.tile_pool(name="stageB2", bufs=2) as sb2,
        tc.tile_pool(name="stageC", bufs=2) as sc_,
        tc.tile_pool(name="stageD", bufs=1) as sd_,
        tc.tile_pool(name="probs", bufs=3) as sdp,
        tc.tile_pool(name="dsmall", bufs=2) as sds,
        tc.tile_pool(name="psC", bufs=2, space="PSUM") as psc,
        tc.tile_pool(name="psSC", bufs=2, space="PSUM") as ps_sc,
        tc.tile_pool(name="psAT", bufs=2, space="PSUM") as ps_at,
    ):
        # ---------------- stage C: ca transpose + kv projections ----------------
        ca_view = ca_d.ap().rearrange("(c t p) d -> c p t d", p=128, t=TPC)

        def emit_chunk(ch):
            canat = sc_.tile([128, TPC, CA], F32R, tag="canat", name="canat")
            nc.gpsimd.dma_start(canat[:], ca_view[ch])
            caT = sc_.tile([128, NKT, SCH], F32R, tag="caT", name="caT")
            for pt in range(TPC):
                for grp in range(2):
                    tp = psc.tile([128, 512], F32R, tag="cps", name="tp")
                    for jj in range(4):
                        kb = grp * 4 + jj
                        nc.tensor.transpose(tp[:, jj * 128:(jj + 1) * 128],
                                            canat[:, pt, kb * 128:(kb + 1) * 128],
                                            ident_r[:])
                    nc.vector.tensor_copy(
                        caT[:, grp * 4:(grp + 1) * 4, pt * 128:(pt + 1) * 128],
                        tp[:].rearrange("p (a b) -> p a b", a=4))
            for m in range(2):
                kp = psc.tile([128, SCH], F32, tag="cps", name="kp")
                for kt in range(NKT):
                    nc.tensor.matmul(kp[:],
                                     wk_r[:, kt, m * 128:(m + 1) * 128],
                                     caT[:, kt, :],
                                     start=(kt == 0), stop=(kt == NKT - 1))
                nc.vector.tensor_copy(kT[ch][:, m, :], kp[:])
            for st in range(TPC):
                vp = psc.tile([128, CS], F32, tag="cps", name="vp")
                for kt in range(NKT):
                    nc.tensor.matmul(vp[:],
                                     caT[:, kt, st * 128:(st + 1) * 128],
                                     wv_r[:, kt, :],
                                     start=(kt == 0), stop=(kt == NKT - 1))
                nc.vector.tensor_copy(
                    v_sb[ch][:, st, :].rearrange("p (h e) -> p h e", h=HPC)[:, :, 0:64],
                    vp[:].rearrange("p (h e) -> p h e", h=HPC))

        for ch in range(3):
            emit_chunk(ch)

        # ---------------- stage B: x-path (norm, transpose, q) ----------------
        x_view = x_d.ap().rearrange("(k p) c -> p k c", p=128)
        xnT = sb_.tile([128, NKT, 512], F32R, tag="xnT")
        ss = sb_.tile([128, NLT], F32, tag="ss")
        nrm = sb_.tile([128, NLT * 4], F32, tag="nrm")
        for lh in range(2):
            for j in range(4):
                lt = lh * 4 + j
                x_t = sb2.tile([128, D], F32, tag="x_t")
                nc.sync.dma_start(x_t[:], x_view[:, lt, :])
                sq_scratch = sb2.tile([128, D], BF16, tag="sq", bufs=1)
                nc.scalar.activation(sq_scratch[:], x_t[:], AF.Square,
                                     accum_out=ss[:, lt:lt + 1])
                r0 = nrm[:, 4 * lt + 0:4 * lt + 1]
                inv_r = nrm[:, 4 * lt + 1:4 * lt + 2]
                t2 = nrm[:, 4 * lt + 2:4 * lt + 3]
                inv_den = nrm[:, 4 * lt + 3:4 * lt + 4]
                nc.scalar.sqrt(r0, ss[:, lt:lt + 1])
                nc.vector.reciprocal(inv_r, r0)
                nc.vector.tensor_mul(t2, ss[:, lt:lt + 1], inv_r)
                nc.vector.tensor_add(t2, t2, r0)
                nc.vector.tensor_scalar(t2, t2, 0.5 / 32.0, 1.0e-8,
                                        ALU.mult, ALU.add)
                nc.vector.reciprocal(inv_den, t2)
                xn = sb2.tile([128, D], F32R, tag="xn")
                nc.vector.tensor_scalar_mul(xn[:], x_t[:], inv_den)
                for grp in range(2):
                    tp = psc.tile([128, 512], F32R, tag="cps", name="tpB")
                    for jj in range(4):
                        kb = grp * 4 + jj
                        nc.tensor.transpose(tp[:, jj * 128:(jj + 1) * 128],
                                            xn[:, kb * 128:(kb + 1) * 128],
                                            ident_r[:])
                    nc.vector.tensor_copy(
                        xnT[:, grp * 4:(grp + 1) * 4, j * 128:(j + 1) * 128],
                        tp[:].rearrange("p (a b) -> p a b", a=4))
            for m in range(2):
                qp = psc.tile([128, 512], F32, tag="cps", name="qps")
                for kt in range(NKT):
                    nc.tensor.matmul(qp[:],
                                     wq_r[:, kt, m * 128:(m + 1) * 128],
                                     xnT[:, kt, :],
                                     start=(kt == 0), stop=(kt == NKT - 1))
                nc.vector.tensor_copy(qT[:, m, lh * 512:(lh + 1) * 512], qp[:])
        if "qT" in dbg:
            nc.sync.dma_start(dbg["qT"].ap(), qT[:])

        for ch in range(3, NCH):
            emit_chunk(ch)

        # ---------------- stage D: attention ----------------
        # wo load here: DMA engines are past the input-load burst by now
        nc.gpsimd.dma_start(wo_bf[:], wo_d.ap().rearrange("(k p) c -> p k c", p=128))
        attn_n = [sd_.tile([64, L], BF16, tag=f"attn_n{h}", name=f"attn_n{h}")
                  for h in range(HPC)]
        for lc in range(2):
            for hp in range(2):
                at = [ps_at.tile([65, 512], F32, tag="attn", name=f"at{i}")
                      for i in range(2)]
                for t in range(NST):
                    scps = ps_sc.tile([128, 1024], F32, tag="sc")
                    for i in range(2):
                        nc.tensor.matmul(
                            scps[:, i * 512:(i + 1) * 512],
                            kT[t // TPC][i * 64:(i + 1) * 64, hp,
                                         (t % TPC) * 128:(t % TPC + 1) * 128],
                            qT[i * 64:(i + 1) * 64, hp, lc * 512:(lc + 1) * 512],
                            start=True, stop=True)
                    probs = sdp.tile([128, 1024], BF16, tag="probs", bufs=3)
                    nc.scalar.activation(probs[:], scps[:], AF.Exp,
                                         bias=prep_T[:, t:t + 1], scale=1.0)
                    if lc == 0 and hp == 0 and t == 0 and "probs" in dbg:
                        nc.sync.dma_start(dbg["probs"].ap(), probs[:])
                    for i in range(2):
                        h = hp * 2 + i
                        nc.tensor.matmul(
                            at[i],
                            v_sb[t // TPC][:, t % TPC, h * 65:(h + 1) * 65],
                            probs[:, i * 512:(i + 1) * 512],
                            start=(t == 0), stop=(t == NST - 1))
                # copy raw accumulators out of PSUM immediately; normalize
                # from SBUF off the critical path
                raw = [sds.tile([65, 512], F32, tag="rawat", bufs=4,
                                name=f"raw{i}") for i in range(2)]
                for i in range(2):
                    nc.vector.tensor_copy(raw[i][:], at[i][:])
                dinv = sds.tile([128, 1024], F32R, tag="dinv", bufs=1)
                with nc.allow_low_precision(reason="f32r rounding of 1/D is benign"):
                    for i in range(2):
                        nc.vector.reciprocal(dinv[64:65, i * 512:(i + 1) * 512],
                                             raw[i][64:65, :])
                for i in range(2):
                    h = hp * 2 + i
                    dbc = ps_sc.tile([64, 512], F32, tag="sc", name="dbc")
                    nc.tensor.matmul(dbc[:], ones_r[64:65, 0:64],
                                     dinv[64:65, i * 512:(i + 1) * 512],
                                     tile_position=(64, 0),
                                     start=True, stop=True)
                    dbc_sb = sds.tile([64, 512], F32, tag="dbc_sb")
                    nc.vector.tensor_copy(dbc_sb[:], dbc[:])
                    nc.vector.tensor_mul(
                        attn_n[h][:, lc * 512:(lc + 1) * 512],
                        raw[i][0:64, :], dbc_sb[:])
        # A2A staging: shard j goes to core j; the other batch's shards are
        # zeroed via bsel so the receiver can just sum both halves.
        for h in range(HPC):
            a2a_st = sds.tile([64, N_CORES, LSL], BF16, tag="a2a_st", bufs=1,
                              name=f"a2a_st{h}")
            for half in range(2):
                nc.vector.tensor_scalar_mul(
                    a2a_st[:, half * G:(half + 1) * G, :],
                    attn_n[h][:].rearrange("p (j l) -> p j l", j=G),
                    bsel_sb[:, half:half + 1])
            nc.gpsimd.dma_start(
                a2a_in[:, h * 64:(h + 1) * 64, :].rearrange("j p l -> p j l"),
                a2a_st[:])
        if "attn0" in dbg:
            nc.sync.dma_start(dbg["attn0"].ap(), attn_n[0][:])

    nc.gpsimd.collective_compute(
        "AllToAll", ALU.bypass,
        replica_groups=[list(range(N_CORES))],
        ins=[a2a_in.opt()], outs=[a2a_out.opt()])

    # ---------------- stage E: final projection ----------------
    with (
        tc.tile_pool(name="stageE", bufs=1) as se_,
        tc.tile_pool(name="stageE2", bufs=2) as se2,
        tc.tile_pool(name="psE", bufs=2, space="PSUM") as pse,
    ):
        attnTa = se_.tile([128, NKT, LSL], BF16, tag="attnTa")
        attnTb = se_.tile([128, NKT, LSL], BF16, tag="attnTb")
        nc.gpsimd.dma_start(
            attnTa[:],
            a2a_out[0:G].rearrange("i (t p) l -> p (i t) l", p=128))
        nc.gpsimd.dma_start(
            attnTb[:],
            a2a_out[G:2 * G].rearrange("i (t p) l -> p (i t) l", p=128))
        attnT = se_.tile([128, NKT, LSL], BF16, tag="attnT")
        nc.vector.tensor_add(attnT[:], attnTa[:], attnTb[:])
        if "a2a" in dbg:
            nc.sync.dma_start(dbg["a2a"].ap(), a2a_out[:])
        for mt in range(2):
            ysb = se2.tile([128, D], F32, tag="ysb")
            for en in range(2):
                yp = pse.tile([128, 512], F32, tag="yps")
                for kt in range(NKT):
                    nc.tensor.matmul(yp[:],
                                     attnT[:, kt, mt * 128:(mt + 1) * 128],
                                     wo_bf[:, kt, en * 512:(en + 1) * 512],
                                     start=(kt == 0), stop=(kt == NKT - 1))
                nc.vector.tensor_copy(ysb[:, en * 512:(en + 1) * 512], yp[:])
            nc.gpsimd.dma_start(y_d.ap()[mt * 128:(mt + 1) * 128, :], ysb[:])


_NC_CACHE = {}


def _get_nc(repeat=1):
    if repeat not in _NC_CACHE:
        _NC_CACHE[repeat] = build_program(repeat)
    return _NC_CACHE[repeat]


def make_in_maps(inputs):
    x = np.ascontiguousarray(np.asarray(inputs["hidden_states"], dtype=np.float32))
    ca = np.ascontiguousarray(np.asarray(inputs["ca_hidden_states"], dtype=np.float32))
    mask = np.asarray(inputs["ca_attention_mask"], dtype=np.float32)
    scale = np.asarray(inputs["scale"], dtype=np.float32)
    Wq = np.asarray(inputs["Wq"], dtype=np.float32)
    Wkv = np.asarray(inputs["Wkv"], dtype=np.float32)
    Wo = np.ascontiguousarray(np.asarray(inputs["Wo"], dtype=np.float32))
    in_maps = []
    for c in range(N_CORES):
        b, g = c // G, c % G
        bsel = np.zeros((64, 2), np.float32)
        bsel[:, b] = 1.0
        in_maps.append({
            "bsel": bsel,
            "x": x[b],
            "ca": ca[b],
            "mask": np.ascontiguousarray(mask[b].reshape(32, 128)),
            "scale": np.ascontiguousarray(scale.reshape(NKT, 128)),
            "wq": np.ascontiguousarray(Wq[:, g * CS:(g + 1) * CS]),
            "wk": np.ascontiguousarray(Wkv[:, g * CS:(g + 1) * CS]),
            "wv": np.ascontiguousarray(Wkv[:, CA + g * CS:CA + (g + 1) * CS]),
            "wo": Wo,
        })
    return in_maps


def kernel(**inputs) -> np.ndarray:
    nc = _get_nc(1)
    in_maps = make_in_maps(inputs)
    res = run_bass_kernel_spmd(nc, in_maps, core_ids=list(range(N_CORES)))
    out = np.empty((B, L, D), dtype=np.float32)
    for c in range(N_CORES):
        b, g = c // G, c % G
        out[b, g * LSL:(g + 1) * LSL, :] = res.results[c]["y"]
    return out
